# revision 6
# baseline (speedup 1.0000x reference)
"""Multi-head causal self-attention with RoPE, tensor-parallel over heads
across 8 Trainium2 NeuronCores.

Strategy (Megatron-style TP over heads), v2 — fused single pipeline:
  - Each core owns 2 of the 16 heads: rows [c*256,(c+1)*256) of Wq/Wk/Wv
    and the matching columns of Wo. Host sums the 8 partial outputs
    (replaces the TP all-reduce).
  - All matmul operands are bf16 (fp32 PSUM accumulation): enables the
    compiler's fast-weight-load path, 2x DVE elementwise modes, and
    halves input DMA. Tolerance is 2e-2; bf16 lands ~1e-3.
  - One fused stream per batch: for each 512-token chunk si, compute
    q/k (transposed [d,s] layout, RoPE via signed-permutation matmul +
    elementwise) and v (natural [s,d] layout), then IMMEDIATELY the
    causal-attention q-chunk qc=si it unlocks (kc <= qc available), then
    that q-chunk's output projection and DMA-out. The Tile scheduler
    overlaps attention's scalar-engine exp with the next chunk's
    projection matmuls, keeping the PE dense end-to-end.
  - Softmax denominator: exp tiles pair-summed twice (DVE/GpSimd) then
    partition-reduced by an accumulated ones-matmul (no running max:
    scaled scores are O(5), exp cannot overflow bf16).
  - Causal masking: multiplicative {0,1} bf16 mask on the exp output of
    the 4 diagonal k-tiles of each q-chunk.
"""

import sys

import numpy as np

B, S, DIM = 2, 2048, 2048
NUM_HEADS = 16
HD = 128
N_CORES = 8
HPC = NUM_HEADS // N_CORES  # heads per core
DLOC = HPC * HD             # per-core slice of the model dim
ROPE_BASE = 10000.0
SC = 512                    # token chunk: phase-1 s-chunk == attention q-chunk
KC = 128                    # attention k-chunk

_PROGRAM_CACHE = {}


def _rope_tables_T(seq_len, head_dim):
    # match reference float32 arithmetic: inv_freq over even indices,
    # emb = cat(freqs, freqs); returned transposed [head_dim, seq_len]
    inv_freq = (
        1.0
        / (np.float32(ROPE_BASE)
           ** (np.arange(0, head_dim, 2, dtype=np.float32) / np.float32(head_dim)))
    ).astype(np.float32)
    t = np.arange(seq_len, dtype=np.float32)
    freqs = np.outer(t, inv_freq).astype(np.float32)      # [S, D/2]
    emb = np.concatenate([freqs, freqs], axis=-1)         # [S, D]
    return (
        np.ascontiguousarray(np.cos(emb).astype(np.float32).T),
        np.ascontiguousarray(np.sin(emb).astype(np.float32).T),
    )


def _rot_matrix_T(head_dim):
    # rotated = cat(-x[1::2], x[::2]) = R @ x; return R.T [D, D]
    d2 = head_dim // 2
    R = np.zeros((head_dim, head_dim), dtype=np.float32)
    for dp in range(d2):
        R[dp, 2 * dp + 1] = -1.0
    for dp in range(d2, head_dim):
        R[dp, 2 * (dp - d2)] = 1.0
    return np.ascontiguousarray(R.T)


def _causal_masks01(qch):
    # masks[i][kk, qq] = 1 if 128*i + kk <= qq else 0 (multiplicative,
    # applied to exp(scores) on the 4 diagonal k-chunks of each q-chunk)
    m = np.zeros((4, 128, qch), dtype=np.float32)
    kk = np.arange(128)[:, None]
    qq = np.arange(qch)[None, :]
    for i in range(4):
        m[i] = (128 * i + kk <= qq).astype(np.float32)
    return m


def build_program(b=B, s=S, dim=DIM):
    """Builds the per-core SPMD Bass program (identical on every core)."""
    if "/opt/trn_rl_repo" not in sys.path:
        sys.path.insert(0, "/opt/trn_rl_repo")
    import concourse.bacc as bacc
    import concourse.mybir as mybir
    import concourse.tile as tile

    f32 = mybir.dt.float32
    bf16 = mybir.dt.bfloat16
    EXP = mybir.ActivationFunctionType.Exp

    bs = b * s
    n_din = dim // 128          # contraction chunks for projections
    n_sc_b = s // SC            # token chunks per batch (q-chunks)
    n_kpc = SC // KC            # k-chunks per token chunk (4)
    scale = float(HD) ** -0.5

    nc = bacc.Bacc("TRN2", target_bir_lowering=False, debug=False)

    xT_d = nc.dram_tensor("xT", [dim, bs], bf16, kind="ExternalInput")
    wqT_d = nc.dram_tensor("wqT", [dim, DLOC], bf16, kind="ExternalInput")
    wkT_d = nc.dram_tensor("wkT", [dim, DLOC], bf16, kind="ExternalInput")
    wvT_d = nc.dram_tensor("wvT", [dim, DLOC], bf16, kind="ExternalInput")
    woT_d = nc.dram_tensor("woT", [DLOC, dim], bf16, kind="ExternalInput")
    cosT_d = nc.dram_tensor("cosT", [HD, bs], bf16, kind="ExternalInput")
    sinT_d = nc.dram_tensor("sinT", [HD, bs], bf16, kind="ExternalInput")
    rT_d = nc.dram_tensor("rT", [HD, HD], bf16, kind="ExternalInput")
    ones_d = nc.dram_tensor("ones", [HD, HD], bf16, kind="ExternalInput")
    masks_d = nc.dram_tensor("masks", [4, HD, SC], bf16, kind="ExternalInput")
    out_d = nc.dram_tensor("out", [dim, bs], f32, kind="ExternalOutput")

    with tile.TileContext(nc) as tc:
        with (
            tc.tile_pool(name="persist", bufs=1) as persist,
            tc.tile_pool(name="p1w", bufs=1) as p1w,
            tc.tile_pool(name="p1x", bufs=3) as p1x,
            tc.tile_pool(name="p1t", bufs=2) as p1t,
            tc.tile_pool(name="p2", bufs=8) as p2,
            tc.tile_pool(name="p2l1", bufs=4) as p2l1,
            tc.tile_pool(name="p2l2", bufs=2) as p2l2,
            tc.tile_pool(name="p2r", bufs=2) as p2r,
            tc.tile_pool(name="pu", bufs=3) as pu,
            tc.tile_pool(name="p3", bufs=4) as p3,
            tc.tile_pool(name="ps_qk", bufs=2, space="PSUM") as ps_qk,
            tc.tile_pool(name="ps_sl", bufs=3, space="PSUM") as ps_sl,
            tc.tile_pool(name="ps_o", bufs=2, space="PSUM") as ps_o,
            tc.tile_pool(name="ps_l", bufs=1, space="PSUM") as ps_l,
        ):
            # ---------------- persistent tensors ----------------
            qT = persist.tile([128, HPC, bs], bf16)   # roped q, [d, h, b*s]
            kT = persist.tile([128, HPC, bs], bf16)   # roped k, [d, h, b*s]
            vS = persist.tile([128, bs // KC, DLOC], bf16)  # v natural [s, chunk, d]
            woT_s = persist.tile([128, HPC, dim], bf16)
            cosT_s = persist.tile([128, bs], bf16)
            sinT_s = persist.tile([128, bs], bf16)
            rTs = persist.tile([HD, HD], bf16)
            ones = persist.tile([128, 128], bf16)
            masks_s = persist.tile([128, 4, SC], bf16)

            n_si = b * n_sc_b
            xts = {}

            def load_xt(si):
                t = p1x.tile([128, n_din, SC], bf16, tag="xt")
                src_ = xT_d[:, si * SC : (si + 1) * SC].rearrange(
                    "(c p) s -> p c s", p=128
                )
                nh_ = n_din // 2
                nc.sync.dma_start(out=t[:, :nh_, :], in_=src_[:, :nh_, :])
                nc.sync.dma_start(out=t[:, nh_:, :], in_=src_[:, nh_:, :])
                xts[si] = t

            load_xt(0)

            wq_s = p1w.tile([128, n_din, DLOC], bf16)
            wk_s = p1w.tile([128, n_din, DLOC], bf16)
            wv_s = p1w.tile([128, n_din, DLOC], bf16)
            # DMA order: wq first (first matmuls need it), then RoPE tables
            # (needed ~10us in), then wk/wv, then attention/phase-3 tables
            gw = max(1, n_din // 4)
            for g0 in range(0, n_din, gw):
                nc.sync.dma_start(
                    out=wq_s[:, g0 : g0 + gw, :],
                    in_=wqT_d.rearrange("(c p) m -> p c m", p=128)[
                        :, g0 : g0 + gw, :
                    ],
                )
            load_xt(1)
            nc.sync.dma_start(out=rTs, in_=rT_d[:])
            nc.sync.dma_start(out=cosT_s, in_=cosT_d[:])
            nc.sync.dma_start(out=sinT_s, in_=sinT_d[:])
            for g0 in range(0, n_din, gw):
                for w_t, w_d in ((wk_s, wkT_d), (wv_s, wvT_d)):
                    nc.sync.dma_start(
                        out=w_t[:, g0 : g0 + gw, :],
                        in_=w_d.rearrange("(c p) m -> p c m", p=128)[
                            :, g0 : g0 + gw, :
                        ],
                    )
            nc.sync.dma_start(out=ones, in_=ones_d[:])
            nc.sync.dma_start(out=masks_s, in_=masks_d.rearrange("i p q -> p i q"))
            nc.sync.dma_start(
                out=woT_s, in_=woT_d.rearrange("(h p) n -> p h n", p=128)
            )

            li_cnt = 0  # global alternator for DVE/GpSimd pair-sums

            for bi in range(b):
                for sil in range(n_sc_b):
                    si = bi * n_sc_b + sil
                    s0 = si * SC

                    # ---- projections + RoPE for token chunk si ----
                    if si + 2 < n_si:
                        load_xt(si + 2)
                    xt = xts.pop(si)

                    for w_s, store in ((wq_s, qT), (wk_s, kT)):
                        for h in range(HPC):
                            acc = ps_qk.tile([128, SC], f32, tag="qk")
                            for c in range(n_din):
                                nc.tensor.matmul(
                                    acc,
                                    lhsT=w_s[:, c, h * HD : (h + 1) * HD],
                                    rhs=xt[:, c, :],
                                    start=(c == 0),
                                    stop=(c == n_din - 1),
                                )
                            raw = p1t.tile([128, SC], bf16, tag="raw")
                            nc.scalar.copy(raw, acc)
                            rotp = ps_sl.tile([128, SC], f32, tag="sl", name="rotp")
                            nc.tensor.matmul(
                                rotp, lhsT=rTs, rhs=raw, start=True, stop=True
                            )
                            t1 = p1t.tile([128, SC], bf16, tag="t1")
                            nc.vector.tensor_mul(t1, raw, cosT_s[:, s0 : s0 + SC])
                            t2 = p1t.tile([128, SC], bf16, tag="t2")
                            nc.vector.tensor_mul(t2, rotp, sinT_s[:, s0 : s0 + SC])
                            nc.vector.tensor_add(store[:, h, s0 : s0 + SC], t1, t2)

                    for sub in range(SC // KC):
                        vacc = ps_qk.tile([128, DLOC], f32, tag="qk", name="vacc")
                        for c in range(n_din):
                            nc.tensor.matmul(
                                vacc,
                                lhsT=xt[:, c, sub * KC : (sub + 1) * KC],
                                rhs=wv_s[:, c, :],
                                start=(c == 0),
                                stop=(c == n_din - 1),
                            )
                        if sub % 2 == 0:
                            nc.vector.tensor_copy(vS[:, si * n_kpc + sub, :], vacc)
                        else:
                            nc.scalar.copy(vS[:, si * n_kpc + sub, :], vacc)

                    # ---- causal attention for q-chunk qc = sil ----
                    qc = sil
                    q0 = bi * s + qc * SC
                    nkc = (qc + 1) * n_kpc
                    for h in range(HPC):
                        outp = ps_o.tile([128, SC], f32, tag="o")
                        lrep = ps_l.tile([128, SC], f32, tag="l")
                        prev_pt = None
                        prev_lp1 = None
                        nquad = nkc // 4
                        for kc in range(nkc):
                            k0 = bi * s + kc * KC
                            st = ps_sl.tile([128, SC], f32, tag="sl", name="st")
                            nc.tensor.matmul(
                                st,
                                lhsT=kT[:, h, k0 : k0 + KC],
                                rhs=qT[:, h, q0 : q0 + SC],
                                start=True,
                                stop=True,
                            )
                            pt = p2.tile([128, SC], bf16, tag="pt")
                            nc.scalar.activation(pt, st, EXP, scale=scale)
                            di = kc - (nkc - 4)
                            if di >= 0:
                                # multiplicative {0,1} causal mask on exp output
                                nc.vector.tensor_mul(pt, pt, masks_s[:, di, :])
                            nc.tensor.matmul(
                                outp,
                                lhsT=vS[:, bi * (s // KC) + kc, h * HD : (h + 1) * HD],
                                rhs=pt,
                                start=(kc == 0),
                                stop=(kc == nkc - 1),
                            )
                            # softmax denominator: quad pair-sums on DVE/GpSimd,
                            # partition-reduced by an accumulated ones-matmul
                            if kc % 2 == 1:
                                lp1 = p2l1.tile([128, SC], bf16, tag="lp1")
                                eng = nc.vector if li_cnt % 2 == 0 else nc.gpsimd
                                li_cnt += 1
                                eng.tensor_add(lp1, prev_pt, pt)
                                if kc % 4 == 3:
                                    lp2 = p2l2.tile([128, SC], bf16, tag="lp2")
                                    nc.vector.tensor_add(lp2, prev_lp1, lp1)
                                    qi = kc // 4
                                    nc.tensor.matmul(
                                        lrep,
                                        lhsT=ones,
                                        rhs=lp2,
                                        start=(qi == 0),
                                        stop=(qi == nquad - 1),
                                    )
                                else:
                                    prev_lp1 = lp1
                            prev_pt = pt
                        rec = p2r.tile([128, SC], f32, tag="rec")
                        nc.vector.reciprocal_approx_fast(rec, lrep)
                        if h == 0:
                            ut = pu.tile([128, HPC, SC], bf16, tag="ut")
                        nc.vector.tensor_mul(ut[:, h, :], outp, rec)

                    # ---- output projection for (bi, qc) ----
                    for oc in range(dim // 128):
                        o0 = oc * 128
                        pos = ps_qk.tile([128, SC], f32, tag="qk", name="pos")
                        for h in range(HPC):
                            nc.tensor.matmul(
                                pos,
                                lhsT=woT_s[:, h, o0 : o0 + 128],
                                rhs=ut[:, h, :],
                                start=(h == 0),
                                stop=(h == HPC - 1),
                            )
                        ot = p3.tile([128, SC], f32, tag="ot")
                        if oc % 2 == 0:
                            nc.vector.tensor_copy(ot, pos)
                        else:
                            nc.scalar.copy(ot, pos)
                        nc.sync.dma_start(
                            out=out_d[o0 : o0 + 128, s0 : s0 + SC], in_=ot
                        )

    nc.compile()
    return nc


def make_in_maps(x, Wq, Wk, Wv, Wo, b=B, s=S, dim=DIM, n_cores=N_CORES):
    import ml_dtypes

    bf = ml_dtypes.bfloat16
    bs = b * s
    xT = np.ascontiguousarray(x.reshape(bs, dim).T.astype(bf))
    cosT1, sinT1 = _rope_tables_T(s, HD)
    cosT = np.ascontiguousarray(np.tile(cosT1, (1, b)).astype(bf))
    sinT = np.ascontiguousarray(np.tile(sinT1, (1, b)).astype(bf))
    rT = _rot_matrix_T(HD).astype(bf)
    ones = np.ones((HD, HD), dtype=bf)
    masks = _causal_masks01(SC).astype(bf)
    in_maps = []
    for c in range(n_cores):
        sl = slice(c * DLOC, (c + 1) * DLOC)
        in_maps.append(
            {
                "xT": xT,
                "wqT": np.ascontiguousarray(Wq[sl, :].T.astype(bf)),
                "wkT": np.ascontiguousarray(Wk[sl, :].T.astype(bf)),
                "wvT": np.ascontiguousarray(Wv[sl, :].T.astype(bf)),
                "woT": np.ascontiguousarray(Wo[:, sl].T.astype(bf)),
                "cosT": cosT,
                "sinT": sinT,
                "rT": rT,
                "ones": ones,
                "masks": masks,
            }
        )
    return in_maps


def kernel(x, Wq, Wk, Wv, Wo, _trace=False):
    """Full-input / full-output entry point. Shards over 8 cores internally."""
    if "/opt/trn_rl_repo" not in sys.path:
        sys.path.insert(0, "/opt/trn_rl_repo")
    from concourse.bass_utils import run_bass_kernel_spmd

    x = np.asarray(x, dtype=np.float32)
    Wq, Wk, Wv, Wo = (np.asarray(w, dtype=np.float32) for w in (Wq, Wk, Wv, Wo))

    key = (B, S, DIM)
    if key not in _PROGRAM_CACHE:
        _PROGRAM_CACHE[key] = build_program(B, S, DIM)
    nc = _PROGRAM_CACHE[key]

    in_maps = make_in_maps(x, Wq, Wk, Wv, Wo)
    res = run_bass_kernel_spmd(
        nc, in_maps, core_ids=list(range(N_CORES)), trace=_trace
    )
    kernel.last_results = res
    acc = res.results[0]["out"].astype(np.float32)
    for c in range(1, N_CORES):
        acc = acc + res.results[c]["out"]
    return np.ascontiguousarray(acc.T).reshape(B, S, DIM)


# revision 7
# speedup vs baseline: 1.1869x; 1.1869x over previous
"""Multi-head causal self-attention with RoPE, tensor-parallel over heads
across 8 Trainium2 NeuronCores.

Strategy (Megatron-style TP over heads), v2 — fused single pipeline:
  - Each core owns 2 of the 16 heads: rows [c*256,(c+1)*256) of Wq/Wk/Wv
    and the matching columns of Wo. Host sums the 8 partial outputs
    (replaces the TP all-reduce).
  - All matmul operands are bf16 (fp32 PSUM accumulation): enables the
    compiler's fast-weight-load path, 2x DVE elementwise modes, and
    halves input DMA. Tolerance is 2e-2; bf16 lands ~1e-3.
  - One fused stream per batch: for each 512-token chunk si, compute
    q/k (transposed [d,s] layout, RoPE via signed-permutation matmul +
    elementwise) and v (natural [s,d] layout), then IMMEDIATELY the
    causal-attention q-chunk qc=si it unlocks (kc <= qc available), then
    that q-chunk's output projection and DMA-out. The Tile scheduler
    overlaps attention's scalar-engine exp with the next chunk's
    projection matmuls, keeping the PE dense end-to-end.
  - Softmax denominator: exp tiles pair-summed twice (DVE/GpSimd) then
    partition-reduced by an accumulated ones-matmul (no running max:
    scaled scores are O(5), exp cannot overflow bf16).
  - Causal masking: multiplicative {0,1} bf16 mask on the exp output of
    the 4 diagonal k-tiles of each q-chunk.
"""

import sys

import numpy as np

B, S, DIM = 2, 2048, 2048
NUM_HEADS = 16
HD = 128
N_CORES = 8
HPC = NUM_HEADS // N_CORES  # heads per core
DLOC = HPC * HD             # per-core slice of the model dim
ROPE_BASE = 10000.0
SC = 512                    # token chunk: phase-1 s-chunk == attention q-chunk
KC = 128                    # attention k-chunk

_PROGRAM_CACHE = {}


def _rope_tables_T(seq_len, head_dim):
    # match reference float32 arithmetic: inv_freq over even indices,
    # emb = cat(freqs, freqs); returned transposed [head_dim, seq_len]
    inv_freq = (
        1.0
        / (np.float32(ROPE_BASE)
           ** (np.arange(0, head_dim, 2, dtype=np.float32) / np.float32(head_dim)))
    ).astype(np.float32)
    t = np.arange(seq_len, dtype=np.float32)
    freqs = np.outer(t, inv_freq).astype(np.float32)      # [S, D/2]
    emb = np.concatenate([freqs, freqs], axis=-1)         # [S, D]
    return (
        np.ascontiguousarray(np.cos(emb).astype(np.float32).T),
        np.ascontiguousarray(np.sin(emb).astype(np.float32).T),
    )


def _rot_matrix_T(head_dim):
    # rotated = cat(-x[1::2], x[::2]) = R @ x; return R.T [D, D]
    d2 = head_dim // 2
    R = np.zeros((head_dim, head_dim), dtype=np.float32)
    for dp in range(d2):
        R[dp, 2 * dp + 1] = -1.0
    for dp in range(d2, head_dim):
        R[dp, 2 * (dp - d2)] = 1.0
    return np.ascontiguousarray(R.T)


def _causal_masks01(qch):
    # masks[i][kk, qq] = 1 if 128*i + kk <= qq else 0 (multiplicative,
    # applied to exp(scores) on the 4 diagonal k-chunks of each q-chunk)
    m = np.zeros((4, 128, qch), dtype=np.float32)
    kk = np.arange(128)[:, None]
    qq = np.arange(qch)[None, :]
    for i in range(4):
        m[i] = (128 * i + kk <= qq).astype(np.float32)
    return m


def build_program(b=B, s=S, dim=DIM):
    """Builds the per-core SPMD Bass program (identical on every core)."""
    if "/opt/trn_rl_repo" not in sys.path:
        sys.path.insert(0, "/opt/trn_rl_repo")
    import concourse.bacc as bacc
    import concourse.mybir as mybir
    import concourse.tile as tile

    f32 = mybir.dt.float32
    bf16 = mybir.dt.bfloat16
    EXP = mybir.ActivationFunctionType.Exp

    bs = b * s
    n_din = dim // 128          # contraction chunks for projections
    n_sc_b = s // SC            # token chunks per batch (q-chunks)
    n_kpc = SC // KC            # k-chunks per token chunk (4)
    scale = float(HD) ** -0.5

    nc = bacc.Bacc("TRN2", target_bir_lowering=False, debug=False)

    xT_d = nc.dram_tensor("xT", [dim, bs], bf16, kind="ExternalInput")
    wqT_d = nc.dram_tensor("wqT", [dim, DLOC], bf16, kind="ExternalInput")
    wkT_d = nc.dram_tensor("wkT", [dim, DLOC], bf16, kind="ExternalInput")
    wvT_d = nc.dram_tensor("wvT", [dim, DLOC], bf16, kind="ExternalInput")
    woT_d = nc.dram_tensor("woT", [DLOC, dim], bf16, kind="ExternalInput")
    cosT_d = nc.dram_tensor("cosT", [HD, bs], bf16, kind="ExternalInput")
    sinT_d = nc.dram_tensor("sinT", [HD, bs], bf16, kind="ExternalInput")
    rT_d = nc.dram_tensor("rT", [HD, HD], bf16, kind="ExternalInput")
    ones_d = nc.dram_tensor("ones", [HD, HD], bf16, kind="ExternalInput")
    masks_d = nc.dram_tensor("masks", [4, HD, SC], bf16, kind="ExternalInput")
    out_d = nc.dram_tensor("out", [dim, bs], f32, kind="ExternalOutput")

    with tile.TileContext(nc) as tc:
        with (
            tc.tile_pool(name="persist", bufs=1) as persist,
            tc.tile_pool(name="p1w", bufs=1) as p1w,
            tc.tile_pool(name="p1x", bufs=3) as p1x,
            tc.tile_pool(name="p1t", bufs=2) as p1t,
            tc.tile_pool(name="p2", bufs=8) as p2,
            tc.tile_pool(name="p2l1", bufs=4) as p2l1,
            tc.tile_pool(name="p2l2", bufs=2) as p2l2,
            tc.tile_pool(name="p2r", bufs=2) as p2r,
            tc.tile_pool(name="pu", bufs=3) as pu,
            tc.tile_pool(name="p3", bufs=4) as p3,
            tc.tile_pool(name="ps_qk", bufs=2, space="PSUM") as ps_qk,
            tc.tile_pool(name="ps_sl", bufs=2, space="PSUM") as ps_sl,
            tc.tile_pool(name="ps_o", bufs=3, space="PSUM") as ps_o,
            tc.tile_pool(name="ps_l", bufs=1, space="PSUM") as ps_l,
        ):
            # ---------------- persistent tensors ----------------
            qT = persist.tile([128, HPC, bs], bf16)   # roped q, [d, h, b*s]
            kT = persist.tile([128, HPC, bs], bf16)   # roped k, [d, h, b*s]
            vS = persist.tile([128, bs // KC, DLOC], bf16)  # v natural [s, chunk, d]
            woT_s = persist.tile([128, HPC, dim], bf16)
            cosT_s = persist.tile([128, bs], bf16)
            sinT_s = persist.tile([128, bs], bf16)
            rTs = persist.tile([HD, HD], bf16)
            ones = persist.tile([128, 128], bf16)
            masks_s = persist.tile([128, 4, SC], bf16)

            n_si = b * n_sc_b
            xts = {}

            def load_xt(si):
                t = p1x.tile([128, n_din, SC], bf16, tag="xt")
                src_ = xT_d[:, si * SC : (si + 1) * SC].rearrange(
                    "(c p) s -> p c s", p=128
                )
                nh_ = n_din // 2
                nc.sync.dma_start(out=t[:, :nh_, :], in_=src_[:, :nh_, :])
                nc.sync.dma_start(out=t[:, nh_:, :], in_=src_[:, nh_:, :])
                xts[si] = t

            load_xt(0)

            wq_s = p1w.tile([128, n_din, DLOC], bf16)
            wk_s = p1w.tile([128, n_din, DLOC], bf16)
            wv_s = p1w.tile([128, n_din, DLOC], bf16)
            # DMA order: wq first (first matmuls need it), then RoPE tables
            # (needed ~10us in), then wk/wv, then attention/phase-3 tables
            gw = max(1, n_din // 4)
            for g0 in range(0, n_din, gw):
                nc.sync.dma_start(
                    out=wq_s[:, g0 : g0 + gw, :],
                    in_=wqT_d.rearrange("(c p) m -> p c m", p=128)[
                        :, g0 : g0 + gw, :
                    ],
                )
            load_xt(1)
            nc.sync.dma_start(out=rTs, in_=rT_d[:])
            nc.sync.dma_start(out=cosT_s, in_=cosT_d[:])
            nc.sync.dma_start(out=sinT_s, in_=sinT_d[:])
            for g0 in range(0, n_din, gw):
                for w_t, w_d in ((wk_s, wkT_d), (wv_s, wvT_d)):
                    nc.sync.dma_start(
                        out=w_t[:, g0 : g0 + gw, :],
                        in_=w_d.rearrange("(c p) m -> p c m", p=128)[
                            :, g0 : g0 + gw, :
                        ],
                    )
            nc.sync.dma_start(out=ones, in_=ones_d[:])
            nc.sync.dma_start(out=masks_s, in_=masks_d.rearrange("i p q -> p i q"))
            nc.sync.dma_start(
                out=woT_s, in_=woT_d.rearrange("(h p) n -> p h n", p=128)
            )

            li_cnt = 0  # global alternator for DVE/GpSimd pair-sums

            for bi in range(b):
                for sil in range(n_sc_b):
                    si = bi * n_sc_b + sil
                    s0 = si * SC

                    # ---- projections + RoPE for token chunk si ----
                    if si + 2 < n_si:
                        load_xt(si + 2)
                    xt = xts.pop(si)

                    for w_s, store in ((wq_s, qT), (wk_s, kT)):
                        for h in range(HPC):
                            acc = ps_qk.tile([128, SC], f32, tag="qk")
                            for c in range(n_din):
                                nc.tensor.matmul(
                                    acc,
                                    lhsT=w_s[:, c, h * HD : (h + 1) * HD],
                                    rhs=xt[:, c, :],
                                    start=(c == 0),
                                    stop=(c == n_din - 1),
                                )
                            raw = p1t.tile([128, SC], bf16, tag="raw")
                            nc.scalar.copy(raw, acc)
                            rotp = ps_sl.tile([128, SC], f32, tag="sl", name="rotp")
                            nc.tensor.matmul(
                                rotp, lhsT=rTs, rhs=raw, start=True, stop=True
                            )
                            t1 = p1t.tile([128, SC], bf16, tag="t1")
                            nc.vector.tensor_mul(t1, raw, cosT_s[:, s0 : s0 + SC])
                            t2 = p1t.tile([128, SC], bf16, tag="t2")
                            nc.vector.tensor_mul(t2, rotp, sinT_s[:, s0 : s0 + SC])
                            nc.vector.tensor_add(store[:, h, s0 : s0 + SC], t1, t2)

                    for sub in range(SC // KC):
                        vacc = ps_qk.tile([128, DLOC], f32, tag="qk", name="vacc")
                        for c in range(n_din):
                            nc.tensor.matmul(
                                vacc,
                                lhsT=xt[:, c, sub * KC : (sub + 1) * KC],
                                rhs=wv_s[:, c, :],
                                start=(c == 0),
                                stop=(c == n_din - 1),
                            )
                        if sub % 2 == 0:
                            nc.vector.tensor_copy(vS[:, si * n_kpc + sub, :], vacc)
                        else:
                            nc.scalar.copy(vS[:, si * n_kpc + sub, :], vacc)

                    # ---- causal attention for q-chunk qc = sil ----
                    qc = sil
                    q0 = bi * s + qc * SC
                    nkc = (qc + 1) * n_kpc
                    for h in range(HPC):
                        outp = ps_o.tile([128, SC], f32, tag="o")
                        lrep = ps_l.tile([128, SC], f32, tag="l")
                        prev_pt = None
                        prev_lp1 = None
                        nquad = nkc // 4
                        for kc in range(nkc):
                            k0 = bi * s + kc * KC
                            st = ps_sl.tile([128, SC], f32, tag="sl", name="st")
                            nc.tensor.matmul(
                                st,
                                lhsT=kT[:, h, k0 : k0 + KC],
                                rhs=qT[:, h, q0 : q0 + SC],
                                start=True,
                                stop=True,
                            )
                            pt = p2.tile([128, SC], bf16, tag="pt")
                            nc.scalar.activation(pt, st, EXP, scale=scale)
                            di = kc - (nkc - 4)
                            if di >= 0:
                                # multiplicative {0,1} causal mask on exp output
                                nc.vector.tensor_mul(pt, pt, masks_s[:, di, :])
                            nc.tensor.matmul(
                                outp,
                                lhsT=vS[:, bi * (s // KC) + kc, h * HD : (h + 1) * HD],
                                rhs=pt,
                                start=(kc == 0),
                                stop=(kc == nkc - 1),
                            )
                            # softmax denominator: quad pair-sums on DVE/GpSimd,
                            # partition-reduced by an accumulated ones-matmul
                            if kc % 2 == 1:
                                lp1 = p2l1.tile([128, SC], bf16, tag="lp1")
                                eng = nc.vector if li_cnt % 2 == 0 else nc.gpsimd
                                li_cnt += 1
                                eng.tensor_add(lp1, prev_pt, pt)
                                if kc % 4 == 3:
                                    lp2 = p2l2.tile([128, SC], bf16, tag="lp2")
                                    nc.vector.tensor_add(lp2, prev_lp1, lp1)
                                    qi = kc // 4
                                    nc.tensor.matmul(
                                        lrep,
                                        lhsT=ones,
                                        rhs=lp2,
                                        start=(qi == 0),
                                        stop=(qi == nquad - 1),
                                    )
                                else:
                                    prev_lp1 = lp1
                            prev_pt = pt
                        rec = p2r.tile([128, SC], f32, tag="rec")
                        nc.vector.reciprocal_approx_fast(rec, lrep)
                        if h == 0:
                            ut = pu.tile([128, HPC, SC], bf16, tag="ut")
                        nc.vector.tensor_mul(ut[:, h, :], outp, rec)

                    # ---- output projection for (bi, qc) ----
                    for oc in range(dim // 128):
                        o0 = oc * 128
                        pos = ps_o.tile([128, SC], f32, tag="o", name="pos")
                        for h in range(HPC):
                            nc.tensor.matmul(
                                pos,
                                lhsT=woT_s[:, h, o0 : o0 + 128],
                                rhs=ut[:, h, :],
                                start=(h == 0),
                                stop=(h == HPC - 1),
                            )
                        ot = p3.tile([128, SC], f32, tag="ot")
                        if oc % 2 == 0:
                            nc.vector.tensor_copy(ot, pos)
                        else:
                            nc.scalar.copy(ot, pos)
                        nc.sync.dma_start(
                            out=out_d[o0 : o0 + 128, s0 : s0 + SC], in_=ot
                        )

    nc.compile()
    return nc


def make_in_maps(x, Wq, Wk, Wv, Wo, b=B, s=S, dim=DIM, n_cores=N_CORES):
    import ml_dtypes

    bf = ml_dtypes.bfloat16
    bs = b * s
    xT = np.ascontiguousarray(x.reshape(bs, dim).T.astype(bf))
    cosT1, sinT1 = _rope_tables_T(s, HD)
    cosT = np.ascontiguousarray(np.tile(cosT1, (1, b)).astype(bf))
    sinT = np.ascontiguousarray(np.tile(sinT1, (1, b)).astype(bf))
    rT = _rot_matrix_T(HD).astype(bf)
    ones = np.ones((HD, HD), dtype=bf)
    masks = _causal_masks01(SC).astype(bf)
    in_maps = []
    for c in range(n_cores):
        sl = slice(c * DLOC, (c + 1) * DLOC)
        in_maps.append(
            {
                "xT": xT,
                "wqT": np.ascontiguousarray(Wq[sl, :].T.astype(bf)),
                "wkT": np.ascontiguousarray(Wk[sl, :].T.astype(bf)),
                "wvT": np.ascontiguousarray(Wv[sl, :].T.astype(bf)),
                "woT": np.ascontiguousarray(Wo[:, sl].T.astype(bf)),
                "cosT": cosT,
                "sinT": sinT,
                "rT": rT,
                "ones": ones,
                "masks": masks,
            }
        )
    return in_maps


def kernel(x, Wq, Wk, Wv, Wo, _trace=False):
    """Full-input / full-output entry point. Shards over 8 cores internally."""
    if "/opt/trn_rl_repo" not in sys.path:
        sys.path.insert(0, "/opt/trn_rl_repo")
    from concourse.bass_utils import run_bass_kernel_spmd

    x = np.asarray(x, dtype=np.float32)
    Wq, Wk, Wv, Wo = (np.asarray(w, dtype=np.float32) for w in (Wq, Wk, Wv, Wo))

    key = (B, S, DIM)
    if key not in _PROGRAM_CACHE:
        _PROGRAM_CACHE[key] = build_program(B, S, DIM)
    nc = _PROGRAM_CACHE[key]

    in_maps = make_in_maps(x, Wq, Wk, Wv, Wo)
    res = run_bass_kernel_spmd(
        nc, in_maps, core_ids=list(range(N_CORES)), trace=_trace
    )
    kernel.last_results = res
    acc = res.results[0]["out"].astype(np.float32)
    for c in range(1, N_CORES):
        acc = acc + res.results[c]["out"]
    return np.ascontiguousarray(acc.T).reshape(B, S, DIM)


# revision 8
# speedup vs baseline: 1.2419x; 1.0464x over previous
"""Multi-head causal self-attention with RoPE, tensor-parallel over heads
across 8 Trainium2 NeuronCores.

Strategy (Megatron-style TP over heads), v2 — fused single pipeline:
  - Each core owns 2 of the 16 heads: rows [c*256,(c+1)*256) of Wq/Wk/Wv
    and the matching columns of Wo. Host sums the 8 partial outputs
    (replaces the TP all-reduce).
  - All matmul operands are bf16 (fp32 PSUM accumulation): enables the
    compiler's fast-weight-load path, 2x DVE elementwise modes, and
    halves input DMA. Tolerance is 2e-2; bf16 lands ~1e-3.
  - One fused stream per batch: for each 512-token chunk si, compute
    q/k (transposed [d,s] layout, RoPE via signed-permutation matmul +
    elementwise) and v (natural [s,d] layout), then IMMEDIATELY the
    causal-attention q-chunk qc=si it unlocks (kc <= qc available), then
    that q-chunk's output projection and DMA-out. The Tile scheduler
    overlaps attention's scalar-engine exp with the next chunk's
    projection matmuls, keeping the PE dense end-to-end.
  - Softmax denominator: exp tiles pair-summed twice (DVE/GpSimd) then
    partition-reduced by an accumulated ones-matmul (no running max:
    scaled scores are O(5), exp cannot overflow bf16).
  - Causal masking: multiplicative {0,1} bf16 mask on the exp output of
    the 4 diagonal k-tiles of each q-chunk.
"""

import sys

import numpy as np

B, S, DIM = 2, 2048, 2048
NUM_HEADS = 16
HD = 128
N_CORES = 8
HPC = NUM_HEADS // N_CORES  # heads per core
DLOC = HPC * HD             # per-core slice of the model dim
ROPE_BASE = 10000.0
SC = 512                    # token chunk: phase-1 s-chunk == attention q-chunk
KC = 128                    # attention k-chunk

_PROGRAM_CACHE = {}


def _rope_tables_T(seq_len, head_dim):
    # match reference float32 arithmetic: inv_freq over even indices,
    # emb = cat(freqs, freqs); returned transposed [head_dim, seq_len]
    inv_freq = (
        1.0
        / (np.float32(ROPE_BASE)
           ** (np.arange(0, head_dim, 2, dtype=np.float32) / np.float32(head_dim)))
    ).astype(np.float32)
    t = np.arange(seq_len, dtype=np.float32)
    freqs = np.outer(t, inv_freq).astype(np.float32)      # [S, D/2]
    emb = np.concatenate([freqs, freqs], axis=-1)         # [S, D]
    return (
        np.ascontiguousarray(np.cos(emb).astype(np.float32).T),
        np.ascontiguousarray(np.sin(emb).astype(np.float32).T),
    )


def _rot_matrix_T(head_dim):
    # rotated = cat(-x[1::2], x[::2]) = R @ x; return R.T [D, D]
    d2 = head_dim // 2
    R = np.zeros((head_dim, head_dim), dtype=np.float32)
    for dp in range(d2):
        R[dp, 2 * dp + 1] = -1.0
    for dp in range(d2, head_dim):
        R[dp, 2 * (dp - d2)] = 1.0
    return np.ascontiguousarray(R.T)


def _causal_masks01(qch):
    # masks[i][kk, qq] = 1 if 128*i + kk <= qq else 0 (multiplicative,
    # applied to exp(scores) on the 4 diagonal k-chunks of each q-chunk)
    m = np.zeros((4, 128, qch), dtype=np.float32)
    kk = np.arange(128)[:, None]
    qq = np.arange(qch)[None, :]
    for i in range(4):
        m[i] = (128 * i + kk <= qq).astype(np.float32)
    return m


def build_program(b=B, s=S, dim=DIM):
    """Builds the per-core SPMD Bass program (identical on every core)."""
    if "/opt/trn_rl_repo" not in sys.path:
        sys.path.insert(0, "/opt/trn_rl_repo")
    import concourse.bacc as bacc
    import concourse.mybir as mybir
    import concourse.tile as tile

    f32 = mybir.dt.float32
    bf16 = mybir.dt.bfloat16
    EXP = mybir.ActivationFunctionType.Exp

    bs = b * s
    n_din = dim // 128          # contraction chunks for projections
    n_sc_b = s // SC            # token chunks per batch (q-chunks)
    n_kpc = SC // KC            # k-chunks per token chunk (4)
    scale = float(HD) ** -0.5

    nc = bacc.Bacc("TRN2", target_bir_lowering=False, debug=False)

    xT_d = nc.dram_tensor("xT", [dim, bs], bf16, kind="ExternalInput")
    wqT_d = nc.dram_tensor("wqT", [dim, DLOC], bf16, kind="ExternalInput")
    wkT_d = nc.dram_tensor("wkT", [dim, DLOC], bf16, kind="ExternalInput")
    wvT_d = nc.dram_tensor("wvT", [dim, DLOC], bf16, kind="ExternalInput")
    woT_d = nc.dram_tensor("woT", [DLOC, dim], bf16, kind="ExternalInput")
    cosT_d = nc.dram_tensor("cosT", [HD, bs], bf16, kind="ExternalInput")
    sinT_d = nc.dram_tensor("sinT", [HD, bs], bf16, kind="ExternalInput")
    rT_d = nc.dram_tensor("rT", [HD, HD], bf16, kind="ExternalInput")
    ones_d = nc.dram_tensor("ones", [HD, HD], bf16, kind="ExternalInput")
    masks_d = nc.dram_tensor("masks", [4, HD, SC], bf16, kind="ExternalInput")
    out_d = nc.dram_tensor("out", [dim, bs], f32, kind="ExternalOutput")

    with tile.TileContext(nc) as tc:
        with (
            tc.tile_pool(name="persist", bufs=1) as persist,
            tc.tile_pool(name="p1w", bufs=1) as p1w,
            tc.tile_pool(name="p1x", bufs=3) as p1x,
            tc.tile_pool(name="p1t", bufs=2) as p1t,
            tc.tile_pool(name="p2", bufs=8) as p2,
            tc.tile_pool(name="p2l1", bufs=4) as p2l1,
            tc.tile_pool(name="p2l2", bufs=3) as p2l2,
            tc.tile_pool(name="p2r", bufs=2) as p2r,
            tc.tile_pool(name="pu", bufs=3) as pu,
            tc.tile_pool(name="p3", bufs=4) as p3,
            tc.tile_pool(name="ps_qk", bufs=2, space="PSUM") as ps_qk,
            tc.tile_pool(name="ps_sl", bufs=3, space="PSUM") as ps_sl,
            tc.tile_pool(name="ps_o", bufs=3, space="PSUM") as ps_o,
        ):
            # ---------------- persistent tensors ----------------
            qT = persist.tile([128, HPC, bs], bf16)   # roped q, [d, h, b*s]
            kT = persist.tile([128, HPC, bs], bf16)   # roped k, [d, h, b*s]
            vS = persist.tile([128, bs // KC, DLOC], bf16)  # v natural [s, chunk, d]
            woT_s = persist.tile([128, HPC, dim], bf16)
            cosT_s = persist.tile([128, bs], bf16)
            sinT_s = persist.tile([128, bs], bf16)
            rTs = persist.tile([HD, HD], bf16)
            ones = persist.tile([128, 128], bf16)
            masks_s = persist.tile([128, 4, SC], bf16)

            n_si = b * n_sc_b
            xts = {}

            def load_xt(si, nsplit=2):
                t = p1x.tile([128, n_din, SC], bf16, tag="xt")
                src_ = xT_d[:, si * SC : (si + 1) * SC].rearrange(
                    "(c p) s -> p c s", p=128
                )
                g = n_din // nsplit
                for c0 in range(0, n_din, g):
                    nc.sync.dma_start(
                        out=t[:, c0 : c0 + g, :], in_=src_[:, c0 : c0 + g, :]
                    )
                xts[si] = t

            wq_s = p1w.tile([128, n_din, DLOC], bf16)
            wk_s = p1w.tile([128, n_din, DLOC], bf16)
            wv_s = p1w.tile([128, n_din, DLOC], bf16)
            # DMA order: wq + x(si0) interleaved at c-chunk granularity so the
            # first accumulation matmuls start ~2us in; RoPE tables next (the
            # first rope ops need them); wk/wv; x(si1); attention tables last
            gw = max(1, n_din // 4)
            load_xt(0, nsplit=4)
            for g0 in range(0, n_din, gw):
                nc.sync.dma_start(
                    out=wq_s[:, g0 : g0 + gw, :],
                    in_=wqT_d.rearrange("(c p) m -> p c m", p=128)[
                        :, g0 : g0 + gw, :
                    ],
                )
            nc.sync.dma_start(out=rTs, in_=rT_d[:])
            nc.sync.dma_start(out=cosT_s, in_=cosT_d[:])
            nc.sync.dma_start(out=sinT_s, in_=sinT_d[:])
            for g0 in range(0, n_din, gw):
                for w_t, w_d in ((wk_s, wkT_d), (wv_s, wvT_d)):
                    nc.sync.dma_start(
                        out=w_t[:, g0 : g0 + gw, :],
                        in_=w_d.rearrange("(c p) m -> p c m", p=128)[
                            :, g0 : g0 + gw, :
                        ],
                    )
            load_xt(1)
            nc.sync.dma_start(out=ones, in_=ones_d[:])
            nc.sync.dma_start(out=masks_s, in_=masks_d.rearrange("i p q -> p i q"))
            nc.sync.dma_start(
                out=woT_s, in_=woT_d.rearrange("(h p) n -> p h n", p=128)
            )

            li_cnt = 0  # global alternator for DVE/GpSimd pair-sums

            for bi in range(b):
                for sil in range(n_sc_b):
                    si = bi * n_sc_b + sil
                    s0 = si * SC

                    # ---- projections + RoPE for token chunk si ----
                    if si + 2 < n_si:
                        load_xt(si + 2)
                    xt = xts.pop(si)

                    for w_s, store in ((wq_s, qT), (wk_s, kT)):
                        for h in range(HPC):
                            acc = ps_qk.tile([128, SC], f32, tag="qk")
                            for c in range(n_din):
                                nc.tensor.matmul(
                                    acc,
                                    lhsT=w_s[:, c, h * HD : (h + 1) * HD],
                                    rhs=xt[:, c, :],
                                    start=(c == 0),
                                    stop=(c == n_din - 1),
                                )
                            raw = p1t.tile([128, SC], bf16, tag="raw")
                            nc.scalar.copy(raw, acc)
                            rotp = ps_sl.tile([128, SC], f32, tag="sl", name="rotp")
                            nc.tensor.matmul(
                                rotp, lhsT=rTs, rhs=raw, start=True, stop=True
                            )
                            t1 = p1t.tile([128, SC], bf16, tag="t1")
                            nc.vector.tensor_mul(t1, raw, cosT_s[:, s0 : s0 + SC])
                            t2 = p1t.tile([128, SC], bf16, tag="t2")
                            nc.vector.tensor_mul(t2, rotp, sinT_s[:, s0 : s0 + SC])
                            nc.vector.tensor_add(store[:, h, s0 : s0 + SC], t1, t2)

                    for sub in range(SC // KC):
                        vacc = ps_qk.tile([128, DLOC], f32, tag="qk", name="vacc")
                        for c in range(n_din):
                            nc.tensor.matmul(
                                vacc,
                                lhsT=xt[:, c, sub * KC : (sub + 1) * KC],
                                rhs=wv_s[:, c, :],
                                start=(c == 0),
                                stop=(c == n_din - 1),
                            )
                        if sub % 2 == 0:
                            nc.vector.tensor_copy(vS[:, si * n_kpc + sub, :], vacc)
                        else:
                            nc.scalar.copy(vS[:, si * n_kpc + sub, :], vacc)

                    # ---- causal attention for q-chunk qc = sil ----
                    qc = sil
                    q0 = bi * s + qc * SC
                    nkc = (qc + 1) * n_kpc
                    for h in range(HPC):
                        outp = ps_o.tile([128, SC], f32, tag="o")
                        prev_pt = None
                        prev_lp1 = None
                        lsum = None
                        for kc in range(nkc):
                            k0 = bi * s + kc * KC
                            st = ps_sl.tile([128, SC], f32, tag="sl", name="st")
                            nc.tensor.matmul(
                                st,
                                lhsT=kT[:, h, k0 : k0 + KC],
                                rhs=qT[:, h, q0 : q0 + SC],
                                start=True,
                                stop=True,
                            )
                            pt = p2.tile([128, SC], bf16, tag="pt")
                            nc.scalar.activation(pt, st, EXP, scale=scale)
                            di = kc - (nkc - 4)
                            if di >= 0:
                                # multiplicative {0,1} causal mask on exp output
                                nc.vector.tensor_mul(pt, pt, masks_s[:, di, :])
                            nc.tensor.matmul(
                                outp,
                                lhsT=vS[:, bi * (s // KC) + kc, h * HD : (h + 1) * HD],
                                rhs=pt,
                                start=(kc == 0),
                                stop=(kc == nkc - 1),
                            )
                            # softmax denominator: quad pair-sums on DVE/GpSimd,
                            # partition-reduced by an accumulated ones-matmul
                            if kc % 2 == 1:
                                lp1 = p2l1.tile([128, SC], bf16, tag="lp1")
                                eng = nc.vector if li_cnt % 2 == 0 else nc.gpsimd
                                li_cnt += 1
                                eng.tensor_add(lp1, prev_pt, pt)
                                if kc % 4 == 3:
                                    if lsum is None:
                                        lsum = p2l2.tile(
                                            [128, SC], bf16, tag="lp2"
                                        )
                                        nc.vector.tensor_add(lsum, prev_lp1, lp1)
                                    else:
                                        lp2 = p2l2.tile([128, SC], bf16, tag="lp2")
                                        nc.vector.tensor_add(lp2, prev_lp1, lp1)
                                        nc.vector.tensor_add(lsum, lsum, lp2)
                                else:
                                    prev_lp1 = lp1
                            prev_pt = pt
                        lrep = ps_o.tile([128, SC], f32, tag="o", name="lrep")
                        nc.tensor.matmul(
                            lrep, lhsT=ones, rhs=lsum, start=True, stop=True
                        )
                        rec = p2r.tile([128, SC], f32, tag="rec")
                        nc.vector.reciprocal_approx_fast(rec, lrep)
                        if h == 0:
                            ut = pu.tile([128, HPC, SC], bf16, tag="ut")
                        nc.vector.tensor_mul(ut[:, h, :], outp, rec)

                    # ---- output projection for (bi, qc) ----
                    for oc in range(dim // 128):
                        o0 = oc * 128
                        pos = ps_o.tile([128, SC], f32, tag="o", name="pos")
                        for h in range(HPC):
                            nc.tensor.matmul(
                                pos,
                                lhsT=woT_s[:, h, o0 : o0 + 128],
                                rhs=ut[:, h, :],
                                start=(h == 0),
                                stop=(h == HPC - 1),
                            )
                        ot = p3.tile([128, SC], f32, tag="ot")
                        if oc % 2 == 0:
                            nc.vector.tensor_copy(ot, pos)
                        else:
                            nc.scalar.copy(ot, pos)
                        nc.sync.dma_start(
                            out=out_d[o0 : o0 + 128, s0 : s0 + SC], in_=ot
                        )

    nc.compile()
    return nc


def make_in_maps(x, Wq, Wk, Wv, Wo, b=B, s=S, dim=DIM, n_cores=N_CORES):
    import ml_dtypes

    bf = ml_dtypes.bfloat16
    bs = b * s
    xT = np.ascontiguousarray(x.reshape(bs, dim).T.astype(bf))
    cosT1, sinT1 = _rope_tables_T(s, HD)
    cosT = np.ascontiguousarray(np.tile(cosT1, (1, b)).astype(bf))
    sinT = np.ascontiguousarray(np.tile(sinT1, (1, b)).astype(bf))
    rT = _rot_matrix_T(HD).astype(bf)
    ones = np.ones((HD, HD), dtype=bf)
    masks = _causal_masks01(SC).astype(bf)
    in_maps = []
    for c in range(n_cores):
        sl = slice(c * DLOC, (c + 1) * DLOC)
        in_maps.append(
            {
                "xT": xT,
                "wqT": np.ascontiguousarray(Wq[sl, :].T.astype(bf)),
                "wkT": np.ascontiguousarray(Wk[sl, :].T.astype(bf)),
                "wvT": np.ascontiguousarray(Wv[sl, :].T.astype(bf)),
                "woT": np.ascontiguousarray(Wo[:, sl].T.astype(bf)),
                "cosT": cosT,
                "sinT": sinT,
                "rT": rT,
                "ones": ones,
                "masks": masks,
            }
        )
    return in_maps


def kernel(x, Wq, Wk, Wv, Wo, _trace=False):
    """Full-input / full-output entry point. Shards over 8 cores internally."""
    if "/opt/trn_rl_repo" not in sys.path:
        sys.path.insert(0, "/opt/trn_rl_repo")
    from concourse.bass_utils import run_bass_kernel_spmd

    x = np.asarray(x, dtype=np.float32)
    Wq, Wk, Wv, Wo = (np.asarray(w, dtype=np.float32) for w in (Wq, Wk, Wv, Wo))

    key = (B, S, DIM)
    if key not in _PROGRAM_CACHE:
        _PROGRAM_CACHE[key] = build_program(B, S, DIM)
    nc = _PROGRAM_CACHE[key]

    in_maps = make_in_maps(x, Wq, Wk, Wv, Wo)
    res = run_bass_kernel_spmd(
        nc, in_maps, core_ids=list(range(N_CORES)), trace=_trace
    )
    kernel.last_results = res
    acc = res.results[0]["out"].astype(np.float32)
    for c in range(1, N_CORES):
        acc = acc + res.results[c]["out"]
    return np.ascontiguousarray(acc.T).reshape(B, S, DIM)
